# revision 19
# baseline (speedup 1.0000x reference)
"""Trainium2 Bass kernel for nn_AngleFreqEnhance (8-core data-parallel).

Math: out = x + clip(w_out @ Re(IFFT2(gain * FFT2(w_in @ x))), -10, 10)

Key facts exploited:
  * f_enh = (|f|+eps)*gain*exp(i*angle(f)) == gain*f up to O(eps)=1e-8 — the
    frequency step is a pointwise REAL gain multiply.
  * gain depends only on weights_raw (tiny, replicated) — computed on host,
    bit-exact vs the reference via jax-on-CPU (XLA lowers `(t+pi)%pi` to an
    IEEE remainder; numpy does not match it).
  * FFT2/IFFT2 on 128x128 become matmuls with the (symmetric, ortho) DFT
    matrix D: F = D X D. All heavy lifting is TensorEngine bf16 matmuls with
    fp32 PSUM accumulate; rel_l2 error of the whole pipeline ~6e-3.

Per-core dataflow (one sample, B=8 == 8 cores, no collectives):
  DMA-cast x fp32->bf16 (SWDGE) -> proj-in (data-as-weights matmuls,
  out = x_proj^T in [w,(m,h)]) -> per-channel transpose-chain FFT:
      A=D@XT; AT=A^T; B=D@AT=F; Y=gain.*F; C=conj(D)@Y; CT=C^T;
      Zt=Re(conj(D)@CT)=x_enh^T; Z=Zt^T=x_enh
  -> SBUF->SBUF DMA redistribution to [m=16, pixels] -> proj-out ->
  ACT drain-cast -> DVE clip -> DVE residual add into x_bf16 ->
  SWDGE DMA-cast bf16->fp32 out.
"""
import math

import numpy as np
import ml_dtypes

import concourse.bass as bass
import concourse.bacc as bacc
import concourse.mybir as mybir
import concourse.tile as tile
from concourse.bass_utils import run_bass_kernel_spmd

B, C, H, W = 8, 256, 128, 128
M = 16                    # mid channels
HW = H * W                # 16384
N_CORES = 8
OUT_CLIP = 10.0
EPS = 1e-8
N_ANGLES, RADIUS_WIDTH, N_RADII = 8, 8, 9
OVERLAP, HF_RATIO = 1.5, 0.3

F32 = mybir.dt.float32
BF16 = mybir.dt.bfloat16
BF = ml_dtypes.bfloat16

NG = 4                    # channel groups for the FFT chain (4 ch = 512 cols)
GW = (M // NG) * H        # free width of one group = 512


def _build_gain(weights_raw: np.ndarray) -> np.ndarray:
    """Replicates reference gain computation bit-exactly (jax on CPU), returns
    [128, M*128] fp32 laid out [k_h, (m, k_w)] in UNSHIFTED freq coords."""
    import jax
    import jax.numpy as jnp

    cpu = jax.devices("cpu")[0]
    with jax.default_device(cpu):
        cy, cx = H // 2, W // 2
        yy = jnp.arange(H, dtype=jnp.float32)[:, None] - cy
        xx = jnp.arange(W, dtype=jnp.float32)[None, :] - cx
        r = jnp.sqrt(yy * yy + xx * xx)
        theta = (jnp.arctan2(yy, xx) + math.pi) % math.pi
        radius_idx = jnp.clip(
            jnp.floor(r / RADIUS_WIDTH).astype(jnp.int32), 0, N_RADII - 1
        )
        delta = math.pi / N_ANGLES
        half_width = OVERLAP * delta / 2.0
        centers = (jnp.arange(N_ANGLES, dtype=jnp.float32) + 0.5) * delta
        dist = jnp.abs(theta[None, :, :] - centers[:, None, None])
        aw = jnp.clip(1.0 - dist / half_width, 0.0) * (dist < half_width)
        aw = aw / (aw.sum(axis=0, keepdims=True) + EPS)
        max_r = float(max(cy, cx))
        high = (r > HF_RATIO * max_r) if HF_RATIO > 0 else jnp.ones_like(r, dtype=bool)
        valid = (r >= 0.5) & high
        wt = 1.0 + jnp.tanh(jnp.asarray(weights_raw, dtype=jnp.float32))
        w_pix = wt[:, :, radius_idx]
        gain = jnp.einsum("ahw,mahw->mhw", aw, w_pix)
        gain = jnp.where(valid[None], gain, 1.0)
        gain = np.asarray(gain, dtype=np.float32)
    gain = np.fft.ifftshift(gain, axes=(-2, -1))          # [m, kh, kw]
    # per-channel layout [kh, (m, kw)] to match F in the chain
    return np.ascontiguousarray(gain.transpose(1, 0, 2).reshape(H, M * W))


def _dft_mats():
    n = np.arange(H)
    Dc = np.exp(-2j * math.pi * np.outer(n, n) / H) / math.sqrt(H)
    return Dc.real.astype(np.float32), Dc.imag.astype(np.float32)


def _build_nc():
    nc = bacc.Bacc("TRN2", target_bir_lowering=False)

    x_ext = nc.declare_dram_parameter("x", [C, HW], F32, isOutput=False)
    win_ext = nc.declare_dram_parameter("win", [2, 128, M], BF16, isOutput=False)
    wout_ext = nc.declare_dram_parameter("wout", [M, C], BF16, isOutput=False)
    dmat_ext = nc.declare_dram_parameter("dmat", [128, 3 * 128], BF16, isOutput=False)
    ident_ext = nc.declare_dram_parameter("ident", [128, 128], BF16, isOutput=False)
    gain_ext = nc.declare_dram_parameter("gain", [128, M * H], BF16, isOutput=False)
    out_ext = nc.declare_dram_parameter("out", [C, HW], F32, isOutput=True)

    with tile.TileContext(nc) as tc:
        with (
            tc.tile_pool(name="const", bufs=1) as cpool,
            tc.tile_pool(name="big", bufs=1) as bigpool,
            tc.tile_pool(name="work", bufs=3) as wpool,
        ):
            # ---- constants -------------------------------------------------
            dmat = cpool.tile([128, 3 * 128], BF16)
            nc.sync.dma_start(out=dmat[:], in_=dmat_ext[:])
            Dr, Di, Din = dmat[:, 0:128], dmat[:, 128:256], dmat[:, 256:384]
            ident = cpool.tile([128, 128], BF16)
            nc.sync.dma_start(out=ident[:], in_=ident_ext[:])
            win = cpool.tile([128, 2 * M], BF16)
            for ch in range(2):
                nc.sync.dma_start(out=win[:, ch * M:(ch + 1) * M], in_=win_ext[ch])
            wout = cpool.tile([M, C], BF16)
            nc.sync.dma_start(out=wout[:], in_=wout_ext[:])
            gain = cpool.tile([128, M * H], BF16)
            nc.sync.dma_start(out=gain[:], in_=gain_ext[:])

            # ---- big resident buffers -------------------------------------
            xb = bigpool.tile([128, 2 * HW], BF16)      # x in bf16, [c%128, (c//128, hw)]
            xt = bigpool.tile([128, M * H], BF16)       # x_proj^T  [w, (m, h)]
            xe = bigpool.tile([M, HW], BF16)            # x_enh     [m, (h, w)]
            sb = {
                n: bigpool.tile([128, M * H], BF16, name=n, tag=n)
                for n in ("ar", "ai", "atr", "ati", "br", "bi", "yr", "yi",
                          "cr", "ci", "ctr", "cti", "zt", "zb")
            }

            # ---- phase 1: x in (SWDGE cast) + proj-in ----------------------
            for ch in range(2):
                for q in range(8):
                    nc.gpsimd.dma_start(
                        out=xb[:, ch * HW + q * 2048: ch * HW + (q + 1) * 2048],
                        in_=x_ext[ch * 128:(ch + 1) * 128, q * 2048:(q + 1) * 2048],
                    )

            with tc.tile_pool(name="psproj", bufs=1, space="PSUM") as pp:
                ps_proj = pp.tile([128, M * H], F32)     # 4 banks, [w, (h, m)]
                for ch in range(2):
                    for h in range(H):
                        nc.tensor.matmul(
                            ps_proj[:, h * M:(h + 1) * M],
                            lhsT=xb[:, ch * HW + h * W: ch * HW + (h + 1) * W],
                            rhs=win[:, ch * M:(ch + 1) * M],
                            start=(ch == 0 and h % 32 == 0),
                            stop=(ch == 1 and h % 32 == 31),
                            skip_group_check=True,
                        )
                # strided drain: [w,(h,m)] fp32 PSUM -> [w,(m,h)] bf16 SBUF
                nc.vector.tensor_copy(
                    out=xt[:].rearrange("p (m h) -> p m h", m=M),
                    in_=ps_proj[:].rearrange("p (h m) -> p m h", m=M),
                )

            # ---- phase 2: FFT chain (per channel-group of 4) ---------------
            # xt = x_proj^T [w, (m, h)].  Per channel:
            #   A = D @ X^T          [kw, h]   (contract w)
            #   AT = A^T             [h, kw]
            #   B = D @ AT = F       [kh, kw]  (contract h)
            #   Y = gain .* B        [kh, kw]
            #   C = conj(D) @ Y      [h, kw]
            #   CT = C^T             [kw, h]
            #   Zt = Re(conj(D)@CT)  [w, h]    = x_enh^T
            #   Z = Zt^T             [h, w]    = x_enh
            with tc.tile_pool(name="psfft", bufs=3, space="PSUM") as pf:
                for g in range(NG):
                    gs = slice(g * GW, (g + 1) * GW)

                    psa_r = pf.tile([128, GW], F32, tag="psa")
                    psa_i = pf.tile([128, GW], F32, tag="psb")
                    nc.tensor.matmul(psa_r[:], lhsT=Dr, rhs=xt[:, gs])
                    nc.tensor.matmul(psa_i[:], lhsT=Di, rhs=xt[:, gs])
                    nc.scalar.copy(out=sb["ar"][:, gs], in_=psa_r[:])
                    nc.scalar.copy(out=sb["ai"][:, gs], in_=psa_i[:])

                    # transposes A -> AT (bf16 PSUM tiles)
                    pst_r = pf.tile([128, GW], BF16, tag="psa")
                    pst_i = pf.tile([128, GW], BF16, tag="psb")
                    for k in range(4):
                        m0 = g * 4 + k
                        ks = slice(k * 128, (k + 1) * 128)
                        nc.tensor.transpose(
                            pst_r[:, ks], sb["ar"][:, m0 * 128:(m0 + 1) * 128], ident)
                        nc.tensor.transpose(
                            pst_i[:, ks], sb["ai"][:, m0 * 128:(m0 + 1) * 128], ident)
                    nc.vector.tensor_copy(out=sb["atr"][:, gs], in_=pst_r[:])
                    nc.vector.tensor_copy(out=sb["ati"][:, gs], in_=pst_i[:])

                    # B = D @ AT  (= F^T); ACT drains, DVE applies bf16 gain.
                    # Same-weight matmuls adjacent so LDWEIGHTS pipelines.
                    psb_r = pf.tile([128, GW], F32, tag="psa")
                    psb_i = pf.tile([128, GW], F32, tag="psb")
                    nc.tensor.matmul(psb_r[:], lhsT=Dr, rhs=sb["atr"][:, gs],
                                     start=True, stop=False)
                    nc.tensor.matmul(psb_i[:], lhsT=Dr, rhs=sb["ati"][:, gs],
                                     start=True, stop=False)
                    nc.tensor.matmul(psb_i[:], lhsT=Di, rhs=sb["atr"][:, gs],
                                     start=False, stop=True)
                    nc.tensor.matmul(psb_r[:], lhsT=Din, rhs=sb["ati"][:, gs],
                                     start=False, stop=True)
                    nc.scalar.copy(out=sb["br"][:, gs], in_=psb_r[:])
                    nc.scalar.copy(out=sb["bi"][:, gs], in_=psb_i[:])
                    nc.vector.tensor_mul(out=sb["yr"][:, gs], in0=sb["br"][:, gs],
                                         in1=gain[:, gs])
                    nc.vector.tensor_mul(out=sb["yi"][:, gs], in0=sb["bi"][:, gs],
                                         in1=gain[:, gs])

                    # C = conj(D) @ Y
                    psc_r = pf.tile([128, GW], F32, tag="psa")
                    psc_i = pf.tile([128, GW], F32, tag="psb")
                    nc.tensor.matmul(psc_r[:], lhsT=Dr, rhs=sb["yr"][:, gs],
                                     start=True, stop=False)
                    nc.tensor.matmul(psc_i[:], lhsT=Dr, rhs=sb["yi"][:, gs],
                                     start=True, stop=False)
                    nc.tensor.matmul(psc_r[:], lhsT=Di, rhs=sb["yi"][:, gs],
                                     start=False, stop=True)
                    nc.tensor.matmul(psc_i[:], lhsT=Din, rhs=sb["yr"][:, gs],
                                     start=False, stop=True)
                    nc.scalar.copy(out=sb["cr"][:, gs], in_=psc_r[:])
                    nc.scalar.copy(out=sb["ci"][:, gs], in_=psc_i[:])

                    # transposes C -> CT
                    pst2_r = pf.tile([128, GW], BF16, tag="psa")
                    pst2_i = pf.tile([128, GW], BF16, tag="psb")
                    for k in range(4):
                        m0 = g * 4 + k
                        ks = slice(k * 128, (k + 1) * 128)
                        nc.tensor.transpose(
                            pst2_r[:, ks], sb["cr"][:, m0 * 128:(m0 + 1) * 128], ident)
                        nc.tensor.transpose(
                            pst2_i[:, ks], sb["ci"][:, m0 * 128:(m0 + 1) * 128], ident)
                    nc.vector.tensor_copy(out=sb["ctr"][:, gs], in_=pst2_r[:])
                    nc.vector.tensor_copy(out=sb["cti"][:, gs], in_=pst2_i[:])

                    # Zt = Re(conj(D) @ CT) = x_enh^T
                    psz = pf.tile([128, GW], F32, tag="psa")
                    nc.tensor.matmul(psz[:], lhsT=Dr, rhs=sb["ctr"][:, gs],
                                     start=True, stop=False)
                    nc.tensor.matmul(psz[:], lhsT=Di, rhs=sb["cti"][:, gs],
                                     start=False, stop=True)
                    nc.scalar.copy(out=sb["zt"][:, gs], in_=psz[:])

                    # final transpose Zt -> Z = x_enh  [h, (m, w)]
                    psz2 = pf.tile([128, GW], BF16, tag="psb")
                    for k in range(4):
                        m0 = g * 4 + k
                        ks = slice(k * 128, (k + 1) * 128)
                        nc.tensor.transpose(
                            psz2[:, ks], sb["zt"][:, m0 * 128:(m0 + 1) * 128], ident)
                    nc.scalar.copy(out=sb["zb"][:, gs], in_=psz2[:])

                    # redistribute this group's channels right away so the
                    # SBUF->SBUF DMA overlaps the next group's FFT
                    for k in range(4):
                        m0 = g * 4 + k
                        nc.gpsimd.dma_start(
                            out=xe[m0:m0 + 1, :],
                            in_=sb["zb"][:, m0 * W:(m0 + 1) * W],
                        )

            # ---- phase 4: proj-out + clip + residual + out DMA -------------
            # ch0: DVE drains PSUM with fused clip (tensor_scalar dual-op).
            # ch1: ACT drains, GPSIMD clips — balances the three engines.
            with tc.tile_pool(name="psout", bufs=1, space="PSUM") as po:
                for pg in range(8):          # 8 groups x 2048 pixels
                    pgs = slice(pg * 2048, (pg + 1) * 2048)
                    psp = [po.tile([128, 2048], F32, name=f"po{ch}", tag=f"po{ch}")
                           for ch in range(2)]
                    for ch in range(2):
                        for cc in range(4):
                            pc = pg * 4 + cc
                            nc.tensor.matmul(
                                psp[ch][:, cc * 512:(cc + 1) * 512],
                                lhsT=wout[:, ch * 128:(ch + 1) * 128],
                                rhs=xe[:, pc * 512:(pc + 1) * 512],
                            )
                    pbs = []
                    for ch in range(2):
                        pb = wpool.tile([128, 2048], BF16, name=f"pb{ch}",
                                        tag=f"pb{ch}")
                        nc.scalar.copy(out=pb[:], in_=psp[ch][:])
                        nc.vector.tensor_scalar(
                            out=pb[:], in0=pb[:],
                            scalar1=-OUT_CLIP, scalar2=OUT_CLIP,
                            op0=mybir.AluOpType.max, op1=mybir.AluOpType.min,
                        )
                        pbs.append(pb)
                    pb0, pb1 = pbs
                    for ch, pb in ((0, pb0), (1, pb1)):
                        xs = xb[:, ch * HW + pg * 2048: ch * HW + (pg + 1) * 2048]
                        nc.vector.tensor_add(out=xs, in0=xs, in1=pb[:])
                        nc.gpsimd.dma_start(
                            out=out_ext[ch * 128:(ch + 1) * 128, pgs],
                            in_=xs,
                        )
    nc.finalize()
    return nc


_NC = None


def _get_nc():
    global _NC
    if _NC is None:
        _NC = _build_nc()
    return _NC


def _consts(w_in, w_out, weights_raw):
    Drm, Dim = _dft_mats()
    dmat = np.concatenate([Drm, Dim, -Dim], axis=1).astype(BF)
    ident = np.eye(128, dtype=BF)
    win = np.ascontiguousarray(
        np.asarray(w_in, np.float32).T.reshape(2, 128, M)).astype(BF)
    wout = np.ascontiguousarray(np.asarray(w_out, np.float32).T).astype(BF)
    gain = _build_gain(np.asarray(weights_raw, np.float32)).astype(BF)
    return dmat, ident, win, wout, gain


def kernel(x, w_in, w_out, weights_raw):
    x = np.asarray(x, np.float32)
    dmat, ident, win, wout, gain = _consts(w_in, w_out, weights_raw)
    nc = _get_nc()
    in_maps = [
        {
            "x": np.ascontiguousarray(x[b].reshape(C, HW)),
            "win": win,
            "wout": wout,
            "dmat": dmat,
            "ident": ident,
            "gain": gain,
        }
        for b in range(B)
    ]
    res = run_bass_kernel_spmd(nc, in_maps, core_ids=list(range(N_CORES)))
    out = np.stack([np.asarray(res.results[b]["out"], np.float32) for b in range(B)])
    return out.reshape(B, C, H, W)


# revision 20
# speedup vs baseline: 1.0134x; 1.0134x over previous
"""Trainium2 Bass kernel for nn_AngleFreqEnhance (8-core data-parallel).

Math: out = x + clip(w_out @ Re(IFFT2(gain * FFT2(w_in @ x))), -10, 10)

Key facts exploited:
  * f_enh = (|f|+eps)*gain*exp(i*angle(f)) == gain*f up to O(eps)=1e-8 — the
    frequency step is a pointwise REAL gain multiply.
  * gain depends only on weights_raw (tiny, replicated) — computed on host,
    bit-exact vs the reference via jax-on-CPU (XLA lowers `(t+pi)%pi` to an
    IEEE remainder; numpy does not match it).
  * FFT2/IFFT2 on 128x128 become matmuls with the (symmetric, ortho) DFT
    matrix D: F = D X D. All heavy lifting is TensorEngine bf16 matmuls with
    fp32 PSUM accumulate; rel_l2 error of the whole pipeline ~6e-3.

Per-core dataflow (one sample, B=8 == 8 cores, no collectives):
  DMA-cast x fp32->bf16 (SWDGE) -> proj-in (data-as-weights matmuls,
  out = x_proj^T in [w,(m,h)]) -> per-channel transpose-chain FFT:
      A=D@XT; AT=A^T; B=D@AT=F; Y=gain.*F; C=conj(D)@Y; CT=C^T;
      Zt=Re(conj(D)@CT)=x_enh^T; Z=Zt^T=x_enh
  -> SBUF->SBUF DMA redistribution to [m=16, pixels] -> proj-out ->
  ACT drain-cast -> DVE clip -> DVE residual add into x_bf16 ->
  SWDGE DMA-cast bf16->fp32 out.
"""
import math

import numpy as np
import ml_dtypes

import concourse.bass as bass
import concourse.bacc as bacc
import concourse.mybir as mybir
import concourse.tile as tile
from concourse.bass_utils import run_bass_kernel_spmd

B, C, H, W = 8, 256, 128, 128
M = 16                    # mid channels
HW = H * W                # 16384
N_CORES = 8
OUT_CLIP = 10.0
EPS = 1e-8
N_ANGLES, RADIUS_WIDTH, N_RADII = 8, 8, 9
OVERLAP, HF_RATIO = 1.5, 0.3

F32 = mybir.dt.float32
BF16 = mybir.dt.bfloat16
BF = ml_dtypes.bfloat16

NG = 4                    # channel groups for the FFT chain (4 ch = 512 cols)
GW = (M // NG) * H        # free width of one group = 512


def _build_gain(weights_raw: np.ndarray) -> np.ndarray:
    """Replicates reference gain computation bit-exactly (jax on CPU), returns
    [128, M*128] fp32 laid out [k_h, (m, k_w)] in UNSHIFTED freq coords."""
    import jax
    import jax.numpy as jnp

    cpu = jax.devices("cpu")[0]
    with jax.default_device(cpu):
        cy, cx = H // 2, W // 2
        yy = jnp.arange(H, dtype=jnp.float32)[:, None] - cy
        xx = jnp.arange(W, dtype=jnp.float32)[None, :] - cx
        r = jnp.sqrt(yy * yy + xx * xx)
        theta = (jnp.arctan2(yy, xx) + math.pi) % math.pi
        radius_idx = jnp.clip(
            jnp.floor(r / RADIUS_WIDTH).astype(jnp.int32), 0, N_RADII - 1
        )
        delta = math.pi / N_ANGLES
        half_width = OVERLAP * delta / 2.0
        centers = (jnp.arange(N_ANGLES, dtype=jnp.float32) + 0.5) * delta
        dist = jnp.abs(theta[None, :, :] - centers[:, None, None])
        aw = jnp.clip(1.0 - dist / half_width, 0.0) * (dist < half_width)
        aw = aw / (aw.sum(axis=0, keepdims=True) + EPS)
        max_r = float(max(cy, cx))
        high = (r > HF_RATIO * max_r) if HF_RATIO > 0 else jnp.ones_like(r, dtype=bool)
        valid = (r >= 0.5) & high
        wt = 1.0 + jnp.tanh(jnp.asarray(weights_raw, dtype=jnp.float32))
        w_pix = wt[:, :, radius_idx]
        gain = jnp.einsum("ahw,mahw->mhw", aw, w_pix)
        gain = jnp.where(valid[None], gain, 1.0)
        gain = np.asarray(gain, dtype=np.float32)
    gain = np.fft.ifftshift(gain, axes=(-2, -1))          # [m, kh, kw]
    # per-channel layout [kh, (m, kw)] to match F in the chain
    return np.ascontiguousarray(gain.transpose(1, 0, 2).reshape(H, M * W))


def _dft_mats():
    n = np.arange(H)
    Dc = np.exp(-2j * math.pi * np.outer(n, n) / H) / math.sqrt(H)
    return Dc.real.astype(np.float32), Dc.imag.astype(np.float32)


def _build_nc():
    nc = bacc.Bacc("TRN2", target_bir_lowering=False)

    x_ext = nc.declare_dram_parameter("x", [C, HW], F32, isOutput=False)
    win_ext = nc.declare_dram_parameter("win", [2, 128, M], BF16, isOutput=False)
    wout_ext = nc.declare_dram_parameter("wout", [M, C], BF16, isOutput=False)
    dmat_ext = nc.declare_dram_parameter("dmat", [128, 3 * 128], BF16, isOutput=False)
    ident_ext = nc.declare_dram_parameter("ident", [128, 128], BF16, isOutput=False)
    gain_ext = nc.declare_dram_parameter("gain", [128, M * H], BF16, isOutput=False)
    out_ext = nc.declare_dram_parameter("out", [C, HW], F32, isOutput=True)

    with tile.TileContext(nc) as tc:
        with (
            tc.tile_pool(name="const", bufs=1) as cpool,
            tc.tile_pool(name="big", bufs=1) as bigpool,
            tc.tile_pool(name="work", bufs=3) as wpool,
        ):
            # ---- constants -------------------------------------------------
            dmat = cpool.tile([128, 3 * 128], BF16)
            nc.sync.dma_start(out=dmat[:], in_=dmat_ext[:])
            Dr, Di, Din = dmat[:, 0:128], dmat[:, 128:256], dmat[:, 256:384]
            ident = cpool.tile([128, 128], BF16)
            nc.sync.dma_start(out=ident[:], in_=ident_ext[:])
            win = cpool.tile([128, 2 * M], BF16)
            for ch in range(2):
                nc.sync.dma_start(out=win[:, ch * M:(ch + 1) * M], in_=win_ext[ch])
            wout = cpool.tile([M, C], BF16)
            nc.sync.dma_start(out=wout[:], in_=wout_ext[:])
            gain = cpool.tile([128, M * H], BF16)
            nc.sync.dma_start(out=gain[:], in_=gain_ext[:])

            # ---- big resident buffers -------------------------------------
            xb = bigpool.tile([128, 2 * HW], BF16)      # x in bf16, [c%128, (c//128, hw)]
            xt = bigpool.tile([128, M * H], BF16)       # x_proj^T  [w, (m, h)]
            xe = bigpool.tile([M, HW], BF16)            # x_enh     [m, (h, w)]
            sb = {
                n: bigpool.tile([128, M * H], BF16, name=n, tag=n)
                for n in ("ar", "ai", "atr", "ati", "yr", "yi",
                          "cr", "ci", "ctr", "cti", "zt", "zb")
            }

            # ---- phase 1: x in (SWDGE cast) + proj-in ----------------------
            for ch in range(2):
                for q in range(8):
                    nc.gpsimd.dma_start(
                        out=xb[:, ch * HW + q * 2048: ch * HW + (q + 1) * 2048],
                        in_=x_ext[ch * 128:(ch + 1) * 128, q * 2048:(q + 1) * 2048],
                    )

            with tc.tile_pool(name="psproj", bufs=1, space="PSUM") as pp:
                ps_proj = pp.tile([128, M * H], F32)     # 4 banks, [w, (h, m)]
                for ch in range(2):
                    for h in range(H):
                        nc.tensor.matmul(
                            ps_proj[:, h * M:(h + 1) * M],
                            lhsT=xb[:, ch * HW + h * W: ch * HW + (h + 1) * W],
                            rhs=win[:, ch * M:(ch + 1) * M],
                            start=(ch == 0 and h % 32 == 0),
                            stop=(ch == 1 and h % 32 == 31),
                            skip_group_check=True,
                        )
                # strided drain: [w,(h,m)] fp32 PSUM -> [w,(m,h)] bf16 SBUF
                nc.vector.tensor_copy(
                    out=xt[:].rearrange("p (m h) -> p m h", m=M),
                    in_=ps_proj[:].rearrange("p (h m) -> p m h", m=M),
                )

            # ---- phase 2: FFT chain (per channel-group of 4) ---------------
            # xt = x_proj^T [w, (m, h)].  Per channel:
            #   A = D @ X^T          [kw, h]   (contract w)
            #   AT = A^T             [h, kw]
            #   B = D @ AT = F       [kh, kw]  (contract h)
            #   Y = gain .* B        [kh, kw]
            #   C = conj(D) @ Y      [h, kw]
            #   CT = C^T             [kw, h]
            #   Zt = Re(conj(D)@CT)  [w, h]    = x_enh^T
            #   Z = Zt^T             [h, w]    = x_enh
            with tc.tile_pool(name="psfft", bufs=3, space="PSUM") as pf:
                for g in range(NG):
                    gs = slice(g * GW, (g + 1) * GW)

                    psa_r = pf.tile([128, GW], F32, tag="psa")
                    psa_i = pf.tile([128, GW], F32, tag="psb")
                    nc.tensor.matmul(psa_r[:], lhsT=Dr, rhs=xt[:, gs])
                    nc.tensor.matmul(psa_i[:], lhsT=Di, rhs=xt[:, gs])
                    nc.scalar.copy(out=sb["ar"][:, gs], in_=psa_r[:])
                    nc.scalar.copy(out=sb["ai"][:, gs], in_=psa_i[:])

                    # transposes A -> AT (bf16 PSUM tiles)
                    pst_r = pf.tile([128, GW], BF16, tag="psa")
                    pst_i = pf.tile([128, GW], BF16, tag="psb")
                    for k in range(4):
                        m0 = g * 4 + k
                        ks = slice(k * 128, (k + 1) * 128)
                        nc.tensor.transpose(
                            pst_r[:, ks], sb["ar"][:, m0 * 128:(m0 + 1) * 128], ident)
                        nc.tensor.transpose(
                            pst_i[:, ks], sb["ai"][:, m0 * 128:(m0 + 1) * 128], ident)
                    nc.vector.tensor_copy(out=sb["atr"][:, gs], in_=pst_r[:])
                    nc.vector.tensor_copy(out=sb["ati"][:, gs], in_=pst_i[:])

                    # B = D @ AT  (= F^T); ACT drains, DVE applies bf16 gain.
                    # Same-weight matmuls adjacent so LDWEIGHTS pipelines.
                    psb_r = pf.tile([128, GW], F32, tag="psa")
                    psb_i = pf.tile([128, GW], F32, tag="psb")
                    nc.tensor.matmul(psb_r[:], lhsT=Dr, rhs=sb["atr"][:, gs],
                                     start=True, stop=False)
                    nc.tensor.matmul(psb_i[:], lhsT=Dr, rhs=sb["ati"][:, gs],
                                     start=True, stop=False)
                    nc.tensor.matmul(psb_i[:], lhsT=Di, rhs=sb["atr"][:, gs],
                                     start=False, stop=True)
                    nc.tensor.matmul(psb_r[:], lhsT=Din, rhs=sb["ati"][:, gs],
                                     start=False, stop=True)
                    nc.vector.tensor_mul(out=sb["yr"][:, gs], in0=psb_r[:],
                                         in1=gain[:, gs])
                    nc.vector.tensor_mul(out=sb["yi"][:, gs], in0=psb_i[:],
                                         in1=gain[:, gs])

                    # C = conj(D) @ Y
                    psc_r = pf.tile([128, GW], F32, tag="psa")
                    psc_i = pf.tile([128, GW], F32, tag="psb")
                    nc.tensor.matmul(psc_r[:], lhsT=Dr, rhs=sb["yr"][:, gs],
                                     start=True, stop=False)
                    nc.tensor.matmul(psc_i[:], lhsT=Dr, rhs=sb["yi"][:, gs],
                                     start=True, stop=False)
                    nc.tensor.matmul(psc_r[:], lhsT=Di, rhs=sb["yi"][:, gs],
                                     start=False, stop=True)
                    nc.tensor.matmul(psc_i[:], lhsT=Din, rhs=sb["yr"][:, gs],
                                     start=False, stop=True)
                    nc.scalar.copy(out=sb["cr"][:, gs], in_=psc_r[:])
                    nc.scalar.copy(out=sb["ci"][:, gs], in_=psc_i[:])

                    # transposes C -> CT
                    pst2_r = pf.tile([128, GW], BF16, tag="psa")
                    pst2_i = pf.tile([128, GW], BF16, tag="psb")
                    for k in range(4):
                        m0 = g * 4 + k
                        ks = slice(k * 128, (k + 1) * 128)
                        nc.tensor.transpose(
                            pst2_r[:, ks], sb["cr"][:, m0 * 128:(m0 + 1) * 128], ident)
                        nc.tensor.transpose(
                            pst2_i[:, ks], sb["ci"][:, m0 * 128:(m0 + 1) * 128], ident)
                    nc.vector.tensor_copy(out=sb["ctr"][:, gs], in_=pst2_r[:])
                    nc.vector.tensor_copy(out=sb["cti"][:, gs], in_=pst2_i[:])

                    # Zt = Re(conj(D) @ CT) = x_enh^T
                    psz = pf.tile([128, GW], F32, tag="psa")
                    nc.tensor.matmul(psz[:], lhsT=Dr, rhs=sb["ctr"][:, gs],
                                     start=True, stop=False)
                    nc.tensor.matmul(psz[:], lhsT=Di, rhs=sb["cti"][:, gs],
                                     start=False, stop=True)
                    nc.scalar.copy(out=sb["zt"][:, gs], in_=psz[:])

                    # final transpose Zt -> Z = x_enh  [h, (m, w)]
                    psz2 = pf.tile([128, GW], BF16, tag="psb")
                    for k in range(4):
                        m0 = g * 4 + k
                        ks = slice(k * 128, (k + 1) * 128)
                        nc.tensor.transpose(
                            psz2[:, ks], sb["zt"][:, m0 * 128:(m0 + 1) * 128], ident)
                    nc.scalar.copy(out=sb["zb"][:, gs], in_=psz2[:])

                    # redistribute this group's channels right away so the
                    # SBUF->SBUF DMA overlaps the next group's FFT
                    for k in range(4):
                        m0 = g * 4 + k
                        nc.gpsimd.dma_start(
                            out=xe[m0:m0 + 1, :],
                            in_=sb["zb"][:, m0 * W:(m0 + 1) * W],
                        )

            # ---- phase 4: proj-out + clip + residual + out DMA -------------
            # ch0: DVE drains PSUM with fused clip (tensor_scalar dual-op).
            # ch1: ACT drains, GPSIMD clips — balances the three engines.
            with tc.tile_pool(name="psout", bufs=1, space="PSUM") as po:
                for pg in range(8):          # 8 groups x 2048 pixels
                    pgs = slice(pg * 2048, (pg + 1) * 2048)
                    psp = [po.tile([128, 2048], F32, name=f"po{ch}", tag=f"po{ch}")
                           for ch in range(2)]
                    for ch in range(2):
                        for cc in range(4):
                            pc = pg * 4 + cc
                            nc.tensor.matmul(
                                psp[ch][:, cc * 512:(cc + 1) * 512],
                                lhsT=wout[:, ch * 128:(ch + 1) * 128],
                                rhs=xe[:, pc * 512:(pc + 1) * 512],
                            )
                    pbs = []
                    for ch in range(2):
                        pb = wpool.tile([128, 2048], BF16, name=f"pb{ch}",
                                        tag=f"pb{ch}")
                        nc.scalar.copy(out=pb[:], in_=psp[ch][:])
                        nc.vector.tensor_scalar(
                            out=pb[:], in0=pb[:],
                            scalar1=-OUT_CLIP, scalar2=OUT_CLIP,
                            op0=mybir.AluOpType.max, op1=mybir.AluOpType.min,
                        )
                        pbs.append(pb)
                    pb0, pb1 = pbs
                    for ch, pb in ((0, pb0), (1, pb1)):
                        xs = xb[:, ch * HW + pg * 2048: ch * HW + (pg + 1) * 2048]
                        nc.vector.tensor_add(out=xs, in0=xs, in1=pb[:])
                        nc.gpsimd.dma_start(
                            out=out_ext[ch * 128:(ch + 1) * 128, pgs],
                            in_=xs,
                        )
    nc.finalize()
    return nc


_NC = None


def _get_nc():
    global _NC
    if _NC is None:
        _NC = _build_nc()
    return _NC


def _consts(w_in, w_out, weights_raw):
    Drm, Dim = _dft_mats()
    dmat = np.concatenate([Drm, Dim, -Dim], axis=1).astype(BF)
    ident = np.eye(128, dtype=BF)
    win = np.ascontiguousarray(
        np.asarray(w_in, np.float32).T.reshape(2, 128, M)).astype(BF)
    wout = np.ascontiguousarray(np.asarray(w_out, np.float32).T).astype(BF)
    gain = _build_gain(np.asarray(weights_raw, np.float32)).astype(BF)
    return dmat, ident, win, wout, gain


def kernel(x, w_in, w_out, weights_raw):
    x = np.asarray(x, np.float32)
    dmat, ident, win, wout, gain = _consts(w_in, w_out, weights_raw)
    nc = _get_nc()
    in_maps = [
        {
            "x": np.ascontiguousarray(x[b].reshape(C, HW)),
            "win": win,
            "wout": wout,
            "dmat": dmat,
            "ident": ident,
            "gain": gain,
        }
        for b in range(B)
    ]
    res = run_bass_kernel_spmd(nc, in_maps, core_ids=list(range(N_CORES)))
    out = np.stack([np.asarray(res.results[b]["out"], np.float32) for b in range(B)])
    return out.reshape(B, C, H, W)


# revision 21
# speedup vs baseline: 1.1637x; 1.1483x over previous
"""Trainium2 Bass kernel for nn_AngleFreqEnhance (8-core data-parallel).

Math: out = x + clip(w_out @ Re(IFFT2(gain * FFT2(w_in @ x))), -10, 10)

Key facts exploited:
  * f_enh = (|f|+eps)*gain*exp(i*angle(f)) == gain*f up to O(eps)=1e-8 — the
    frequency step is a pointwise REAL gain multiply.
  * gain depends only on weights_raw (tiny, replicated) — computed on host,
    bit-exact vs the reference via jax-on-CPU (XLA lowers `(t+pi)%pi` to an
    IEEE remainder; numpy does not match it).
  * FFT2/IFFT2 on 128x128 become matmuls with the (symmetric, ortho) DFT
    matrix D: F = D X D. All heavy lifting is TensorEngine bf16 matmuls with
    fp32 PSUM accumulate; rel_l2 error of the whole pipeline ~6e-3.

Per-core dataflow (one sample, B=8 == 8 cores, no collectives):
  DMA-cast x fp32->bf16 (SWDGE) -> proj-in (data-as-weights matmuls,
  out = x_proj^T in [w,(m,h)]) -> per-channel transpose-chain FFT:
      A=D@XT; AT=A^T; B=D@AT=F; Y=gain.*F; C=conj(D)@Y; CT=C^T;
      Zt=Re(conj(D)@CT)=x_enh^T; Z=Zt^T=x_enh
  -> SBUF->SBUF DMA redistribution to [m=16, pixels] -> proj-out ->
  ACT drain-cast -> DVE clip -> DVE residual add into x_bf16 ->
  SWDGE DMA-cast bf16->fp32 out.
"""
import math

import numpy as np
import ml_dtypes

import concourse.bass as bass
import concourse.bacc as bacc
import concourse.mybir as mybir
import concourse.tile as tile
from concourse.bass_utils import run_bass_kernel_spmd

B, C, H, W = 8, 256, 128, 128
M = 16                    # mid channels
HW = H * W                # 16384
N_CORES = 8
OUT_CLIP = 10.0
EPS = 1e-8
N_ANGLES, RADIUS_WIDTH, N_RADII = 8, 8, 9
OVERLAP, HF_RATIO = 1.5, 0.3

F32 = mybir.dt.float32
BF16 = mybir.dt.bfloat16
BF = ml_dtypes.bfloat16

NG = 4                    # channel groups for the FFT chain (4 ch = 512 cols)
GW = (M // NG) * H        # free width of one group = 512


def _build_gain(weights_raw: np.ndarray) -> np.ndarray:
    """Replicates reference gain computation bit-exactly (jax on CPU), returns
    [128, M*128] fp32 laid out [k_h, (m, k_w)] in UNSHIFTED freq coords."""
    import jax
    import jax.numpy as jnp

    cpu = jax.devices("cpu")[0]
    with jax.default_device(cpu):
        cy, cx = H // 2, W // 2
        yy = jnp.arange(H, dtype=jnp.float32)[:, None] - cy
        xx = jnp.arange(W, dtype=jnp.float32)[None, :] - cx
        r = jnp.sqrt(yy * yy + xx * xx)
        theta = (jnp.arctan2(yy, xx) + math.pi) % math.pi
        radius_idx = jnp.clip(
            jnp.floor(r / RADIUS_WIDTH).astype(jnp.int32), 0, N_RADII - 1
        )
        delta = math.pi / N_ANGLES
        half_width = OVERLAP * delta / 2.0
        centers = (jnp.arange(N_ANGLES, dtype=jnp.float32) + 0.5) * delta
        dist = jnp.abs(theta[None, :, :] - centers[:, None, None])
        aw = jnp.clip(1.0 - dist / half_width, 0.0) * (dist < half_width)
        aw = aw / (aw.sum(axis=0, keepdims=True) + EPS)
        max_r = float(max(cy, cx))
        high = (r > HF_RATIO * max_r) if HF_RATIO > 0 else jnp.ones_like(r, dtype=bool)
        valid = (r >= 0.5) & high
        wt = 1.0 + jnp.tanh(jnp.asarray(weights_raw, dtype=jnp.float32))
        w_pix = wt[:, :, radius_idx]
        gain = jnp.einsum("ahw,mahw->mhw", aw, w_pix)
        gain = jnp.where(valid[None], gain, 1.0)
        gain = np.asarray(gain, dtype=np.float32)
    gain = np.fft.ifftshift(gain, axes=(-2, -1))          # [m, kh, kw]
    # per-channel layout [kh, (m, kw)] to match F in the chain
    return np.ascontiguousarray(gain.transpose(1, 0, 2).reshape(H, M * W))


def _dft_mats():
    n = np.arange(H)
    Dc = np.exp(-2j * math.pi * np.outer(n, n) / H) / math.sqrt(H)
    return Dc.real.astype(np.float32), Dc.imag.astype(np.float32)


def _build_nc():
    nc = bacc.Bacc("TRN2", target_bir_lowering=False)

    x_ext = nc.declare_dram_parameter("x", [C, HW], F32, isOutput=False)
    win_ext = nc.declare_dram_parameter("win", [2, 128, M], BF16, isOutput=False)
    wout_ext = nc.declare_dram_parameter("wout", [M, C], BF16, isOutput=False)
    dmat_ext = nc.declare_dram_parameter("dmat", [128, 3 * 128], BF16, isOutput=False)
    ident_ext = nc.declare_dram_parameter("ident", [128, 128], BF16, isOutput=False)
    gain_ext = nc.declare_dram_parameter("gain", [128, M * H], BF16, isOutput=False)
    out_ext = nc.declare_dram_parameter("out", [C, HW], F32, isOutput=True)

    with tile.TileContext(nc) as tc:
        with (
            tc.tile_pool(name="const", bufs=1) as cpool,
            tc.tile_pool(name="big", bufs=1) as bigpool,
            tc.tile_pool(name="work", bufs=3) as wpool,
        ):
            # ---- constants -------------------------------------------------
            dmat = cpool.tile([128, 3 * 128], BF16)
            nc.sync.dma_start(out=dmat[:], in_=dmat_ext[:])
            Dr, Di, Din = dmat[:, 0:128], dmat[:, 128:256], dmat[:, 256:384]
            ident = cpool.tile([128, 128], BF16)
            nc.sync.dma_start(out=ident[:], in_=ident_ext[:])
            win = cpool.tile([128, 2 * M], BF16)
            for ch in range(2):
                nc.sync.dma_start(out=win[:, ch * M:(ch + 1) * M], in_=win_ext[ch])
            wout = cpool.tile([M, C], BF16)
            nc.sync.dma_start(out=wout[:], in_=wout_ext[:])
            gain = cpool.tile([128, M * H], BF16)
            nc.sync.dma_start(out=gain[:], in_=gain_ext[:])

            # ---- big resident buffers -------------------------------------
            xb = bigpool.tile([128, 2 * HW], BF16)      # x in bf16, [c%128, (c//128, hw)]
            xt = bigpool.tile([128, M * H], BF16)       # x_proj^T  [w, (m, h)]
            xe = bigpool.tile([M, HW], BF16)            # x_enh     [m, (h, w)]
            sb = {
                n: bigpool.tile([128, M * H], BF16, name=n, tag=n)
                for n in ("ar", "ai", "atr", "ati", "yr", "yi",
                          "cr", "ci", "ctr", "cti", "zt", "zb")
            }

            # ---- phase 1: x in (SWDGE cast) + proj-in ----------------------
            for ch in range(2):
                for q in range(8):
                    nc.gpsimd.dma_start(
                        out=xb[:, ch * HW + q * 2048: ch * HW + (q + 1) * 2048],
                        in_=x_ext[ch * 128:(ch + 1) * 128, q * 2048:(q + 1) * 2048],
                    )

            with tc.tile_pool(name="psproj", bufs=1, space="PSUM") as pp:
                ps_proj = pp.tile([128, M * H], F32)     # 4 banks, [w, (h, m)]
                for ch in range(2):
                    for h in range(H):
                        nc.tensor.matmul(
                            ps_proj[:, h * M:(h + 1) * M],
                            lhsT=xb[:, ch * HW + h * W: ch * HW + (h + 1) * W],
                            rhs=win[:, ch * M:(ch + 1) * M],
                            start=(ch == 0 and h % 32 == 0),
                            stop=(ch == 1 and h % 32 == 31),
                            skip_group_check=True,
                        )
                # strided drain: [w,(h,m)] fp32 PSUM -> [w,(m,h)] bf16 SBUF
                nc.vector.tensor_copy(
                    out=xt[:].rearrange("p (m h) -> p m h", m=M),
                    in_=ps_proj[:].rearrange("p (h m) -> p m h", m=M),
                )

            # ---- phase 2: FFT chain (per channel-group of 4) ---------------
            # xt = x_proj^T [w, (m, h)].  Per channel:
            #   A = D @ X^T          [kw, h]   (contract w)
            #   AT = A^T             [h, kw]
            #   B = D @ AT = F       [kh, kw]  (contract h)
            #   Y = gain .* B        [kh, kw]
            #   C = conj(D) @ Y      [h, kw]
            #   CT = C^T             [kw, h]
            #   Zt = Re(conj(D)@CT)  [w, h]    = x_enh^T
            #   Z = Zt^T             [h, w]    = x_enh
            with tc.tile_pool(name="psfft", bufs=3, space="PSUM") as pf:
                for g in range(NG):
                    gs = slice(g * GW, (g + 1) * GW)

                    psa_r = pf.tile([128, GW], F32, tag="psa")
                    psa_i = pf.tile([128, GW], F32, tag="psb")
                    nc.tensor.matmul(psa_r[:], lhsT=Dr, rhs=xt[:, gs])
                    nc.tensor.matmul(psa_i[:], lhsT=Di, rhs=xt[:, gs])
                    nc.scalar.copy(out=sb["ar"][:, gs], in_=psa_r[:])
                    nc.scalar.copy(out=sb["ai"][:, gs], in_=psa_i[:])

                    # transposes A -> AT (bf16 PSUM tiles)
                    pst_r = pf.tile([128, GW], BF16, tag="psa")
                    pst_i = pf.tile([128, GW], BF16, tag="psb")
                    for k in range(4):
                        m0 = g * 4 + k
                        ks = slice(k * 128, (k + 1) * 128)
                        nc.tensor.transpose(
                            pst_r[:, ks], sb["ar"][:, m0 * 128:(m0 + 1) * 128], ident)
                        nc.tensor.transpose(
                            pst_i[:, ks], sb["ai"][:, m0 * 128:(m0 + 1) * 128], ident)
                    nc.vector.tensor_copy(out=sb["atr"][:, gs], in_=pst_r[:])
                    nc.vector.tensor_copy(out=sb["ati"][:, gs], in_=pst_i[:])

                    # B = D @ AT  (= F^T); ACT drains, DVE applies bf16 gain.
                    # Same-weight matmuls adjacent so LDWEIGHTS pipelines.
                    psb_r = pf.tile([128, GW], F32, tag="psa")
                    psb_i = pf.tile([128, GW], F32, tag="psb")
                    nc.tensor.matmul(psb_r[:], lhsT=Dr, rhs=sb["atr"][:, gs],
                                     start=True, stop=False)
                    nc.tensor.matmul(psb_i[:], lhsT=Dr, rhs=sb["ati"][:, gs],
                                     start=True, stop=False)
                    nc.tensor.matmul(psb_i[:], lhsT=Di, rhs=sb["atr"][:, gs],
                                     start=False, stop=True)
                    nc.tensor.matmul(psb_r[:], lhsT=Din, rhs=sb["ati"][:, gs],
                                     start=False, stop=True)
                    nc.vector.tensor_mul(out=sb["yr"][:, gs], in0=psb_r[:],
                                         in1=gain[:, gs])
                    nc.vector.tensor_mul(out=sb["yi"][:, gs], in0=psb_i[:],
                                         in1=gain[:, gs])

                    # C = conj(D) @ Y
                    psc_r = pf.tile([128, GW], F32, tag="psa")
                    psc_i = pf.tile([128, GW], F32, tag="psb")
                    nc.tensor.matmul(psc_r[:], lhsT=Dr, rhs=sb["yr"][:, gs],
                                     start=True, stop=False)
                    nc.tensor.matmul(psc_i[:], lhsT=Dr, rhs=sb["yi"][:, gs],
                                     start=True, stop=False)
                    nc.tensor.matmul(psc_r[:], lhsT=Di, rhs=sb["yi"][:, gs],
                                     start=False, stop=True)
                    nc.tensor.matmul(psc_i[:], lhsT=Din, rhs=sb["yr"][:, gs],
                                     start=False, stop=True)
                    nc.scalar.copy(out=sb["cr"][:, gs], in_=psc_r[:])
                    nc.scalar.copy(out=sb["ci"][:, gs], in_=psc_i[:])

                    # transposes C -> CT
                    pst2_r = pf.tile([128, GW], BF16, tag="psa")
                    pst2_i = pf.tile([128, GW], BF16, tag="psb")
                    for k in range(4):
                        m0 = g * 4 + k
                        ks = slice(k * 128, (k + 1) * 128)
                        nc.tensor.transpose(
                            pst2_r[:, ks], sb["cr"][:, m0 * 128:(m0 + 1) * 128], ident)
                        nc.tensor.transpose(
                            pst2_i[:, ks], sb["ci"][:, m0 * 128:(m0 + 1) * 128], ident)
                    nc.vector.tensor_copy(out=sb["ctr"][:, gs], in_=pst2_r[:])
                    nc.vector.tensor_copy(out=sb["cti"][:, gs], in_=pst2_i[:])

                    # Zt = Re(conj(D) @ CT) = x_enh^T
                    psz = pf.tile([128, GW], F32, tag="psa")
                    nc.tensor.matmul(psz[:], lhsT=Dr, rhs=sb["ctr"][:, gs],
                                     start=True, stop=False)
                    nc.tensor.matmul(psz[:], lhsT=Di, rhs=sb["cti"][:, gs],
                                     start=False, stop=True)
                    nc.scalar.copy(out=sb["zt"][:, gs], in_=psz[:])

                    # final transpose Zt -> Z = x_enh  [h, (m, w)]
                    psz2 = pf.tile([128, GW], BF16, tag="psb")
                    for k in range(4):
                        m0 = g * 4 + k
                        ks = slice(k * 128, (k + 1) * 128)
                        nc.tensor.transpose(
                            psz2[:, ks], sb["zt"][:, m0 * 128:(m0 + 1) * 128], ident)
                    nc.scalar.copy(out=sb["zb"][:, gs], in_=psz2[:])

                    # redistribute this group's channels right away so the
                    # SBUF->SBUF DMA overlaps the next group's FFT
                    for k in range(4):
                        m0 = g * 4 + k
                        nc.gpsimd.dma_start(
                            out=xe[m0:m0 + 1, :],
                            in_=sb["zb"][:, m0 * W:(m0 + 1) * W],
                        )

            # ---- phase 4: proj-out + clip + residual + out DMA -------------
            # ch0: DVE drains PSUM with fused clip (tensor_scalar dual-op).
            # ch1: ACT drains, GPSIMD clips — balances the three engines.
            with tc.tile_pool(name="psout", bufs=1, space="PSUM") as po:
                for pg in range(8):          # 8 groups x 2048 pixels
                    pgs = slice(pg * 2048, (pg + 1) * 2048)
                    psp = [po.tile([128, 2048], F32, name=f"po{ch}", tag=f"po{ch}")
                           for ch in range(2)]
                    for ch in range(2):
                        for cc in range(4):
                            pc = pg * 4 + cc
                            nc.tensor.matmul(
                                psp[ch][:, cc * 512:(cc + 1) * 512],
                                lhsT=wout[:, ch * 128:(ch + 1) * 128],
                                rhs=xe[:, pc * 512:(pc + 1) * 512],
                            )
                    pb0 = wpool.tile([128, 2048], BF16, tag="pb0")
                    nc.vector.tensor_scalar(
                        out=pb0[:], in0=psp[0][:],
                        scalar1=-OUT_CLIP, scalar2=OUT_CLIP,
                        op0=mybir.AluOpType.max, op1=mybir.AluOpType.min,
                    )
                    pb1 = wpool.tile([128, 2048], BF16, tag="pb1")
                    nc.scalar.copy(out=pb1[:], in_=psp[1][:])
                    nc.vector.tensor_scalar(
                        out=pb1[:], in0=pb1[:],
                        scalar1=-OUT_CLIP, scalar2=OUT_CLIP,
                        op0=mybir.AluOpType.max, op1=mybir.AluOpType.min,
                    )
                    for ch, pb in ((0, pb0), (1, pb1)):
                        xs = xb[:, ch * HW + pg * 2048: ch * HW + (pg + 1) * 2048]
                        nc.vector.tensor_add(out=xs, in0=xs, in1=pb[:])
                        nc.gpsimd.dma_start(
                            out=out_ext[ch * 128:(ch + 1) * 128, pgs],
                            in_=xs,
                        )
    nc.finalize()
    return nc


_NC = None


def _get_nc():
    global _NC
    if _NC is None:
        _NC = _build_nc()
    return _NC


def _consts(w_in, w_out, weights_raw):
    Drm, Dim = _dft_mats()
    dmat = np.concatenate([Drm, Dim, -Dim], axis=1).astype(BF)
    ident = np.eye(128, dtype=BF)
    win = np.ascontiguousarray(
        np.asarray(w_in, np.float32).T.reshape(2, 128, M)).astype(BF)
    wout = np.ascontiguousarray(np.asarray(w_out, np.float32).T).astype(BF)
    gain = _build_gain(np.asarray(weights_raw, np.float32)).astype(BF)
    return dmat, ident, win, wout, gain


def kernel(x, w_in, w_out, weights_raw):
    x = np.asarray(x, np.float32)
    dmat, ident, win, wout, gain = _consts(w_in, w_out, weights_raw)
    nc = _get_nc()
    in_maps = [
        {
            "x": np.ascontiguousarray(x[b].reshape(C, HW)),
            "win": win,
            "wout": wout,
            "dmat": dmat,
            "ident": ident,
            "gain": gain,
        }
        for b in range(B)
    ]
    res = run_bass_kernel_spmd(nc, in_maps, core_ids=list(range(N_CORES)))
    out = np.stack([np.asarray(res.results[b]["out"], np.float32) for b in range(B)])
    return out.reshape(B, C, H, W)


# revision 22
# speedup vs baseline: 1.2673x; 1.0890x over previous
"""Trainium2 Bass kernel for nn_AngleFreqEnhance (8-core data-parallel).

Math: out = x + clip(w_out @ Re(IFFT2(gain * FFT2(w_in @ x))), -10, 10)

Key facts exploited:
  * f_enh = (|f|+eps)*gain*exp(i*angle(f)) == gain*f up to O(eps)=1e-8 — the
    frequency step is a pointwise REAL gain multiply.
  * gain depends only on weights_raw (tiny, replicated) — computed on host,
    bit-exact vs the reference via jax-on-CPU (XLA lowers `(t+pi)%pi` to an
    IEEE remainder; numpy does not match it).
  * FFT2/IFFT2 on 128x128 become matmuls with the (symmetric, ortho) DFT
    matrix D: F = D X D. All heavy lifting is TensorEngine bf16 matmuls with
    fp32 PSUM accumulate; rel_l2 error of the whole pipeline ~6e-3.

Per-core dataflow (one sample, B=8 == 8 cores, no collectives):
  DMA-cast x fp32->bf16 (SWDGE) -> proj-in (data-as-weights matmuls,
  out = x_proj^T in [w,(m,h)]) -> per-channel transpose-chain FFT:
      A=D@XT; AT=A^T; B=D@AT=F; Y=gain.*F; C=conj(D)@Y; CT=C^T;
      Zt=Re(conj(D)@CT)=x_enh^T; Z=Zt^T=x_enh
  -> SBUF->SBUF DMA redistribution to [m=16, pixels] -> proj-out ->
  ACT drain-cast -> DVE clip -> DVE residual add into x_bf16 ->
  SWDGE DMA-cast bf16->fp32 out.
"""
import math

import numpy as np
import ml_dtypes

import concourse.bass as bass
import concourse.bacc as bacc
import concourse.mybir as mybir
import concourse.tile as tile
from concourse.bass_utils import run_bass_kernel_spmd

B, C, H, W = 8, 256, 128, 128
M = 16                    # mid channels
HW = H * W                # 16384
N_CORES = 8
OUT_CLIP = 10.0
EPS = 1e-8
N_ANGLES, RADIUS_WIDTH, N_RADII = 8, 8, 9
OVERLAP, HF_RATIO = 1.5, 0.3

F32 = mybir.dt.float32
BF16 = mybir.dt.bfloat16
BF = ml_dtypes.bfloat16

NG = 4                    # channel groups for the FFT chain (4 ch = 512 cols)
GW = (M // NG) * H        # free width of one group = 512


def _build_gain(weights_raw: np.ndarray) -> np.ndarray:
    """Replicates reference gain computation bit-exactly (jax on CPU), returns
    [128, M*128] fp32 laid out [k_h, (m, k_w)] in UNSHIFTED freq coords."""
    import jax
    import jax.numpy as jnp

    cpu = jax.devices("cpu")[0]
    with jax.default_device(cpu):
        cy, cx = H // 2, W // 2
        yy = jnp.arange(H, dtype=jnp.float32)[:, None] - cy
        xx = jnp.arange(W, dtype=jnp.float32)[None, :] - cx
        r = jnp.sqrt(yy * yy + xx * xx)
        theta = (jnp.arctan2(yy, xx) + math.pi) % math.pi
        radius_idx = jnp.clip(
            jnp.floor(r / RADIUS_WIDTH).astype(jnp.int32), 0, N_RADII - 1
        )
        delta = math.pi / N_ANGLES
        half_width = OVERLAP * delta / 2.0
        centers = (jnp.arange(N_ANGLES, dtype=jnp.float32) + 0.5) * delta
        dist = jnp.abs(theta[None, :, :] - centers[:, None, None])
        aw = jnp.clip(1.0 - dist / half_width, 0.0) * (dist < half_width)
        aw = aw / (aw.sum(axis=0, keepdims=True) + EPS)
        max_r = float(max(cy, cx))
        high = (r > HF_RATIO * max_r) if HF_RATIO > 0 else jnp.ones_like(r, dtype=bool)
        valid = (r >= 0.5) & high
        wt = 1.0 + jnp.tanh(jnp.asarray(weights_raw, dtype=jnp.float32))
        w_pix = wt[:, :, radius_idx]
        gain = jnp.einsum("ahw,mahw->mhw", aw, w_pix)
        gain = jnp.where(valid[None], gain, 1.0)
        gain = np.asarray(gain, dtype=np.float32)
    gain = np.fft.ifftshift(gain, axes=(-2, -1))          # [m, kh, kw]
    # per-channel layout [kh, (m, kw)] to match F in the chain
    return np.ascontiguousarray(gain.transpose(1, 0, 2).reshape(H, M * W))


def _dft_mats():
    n = np.arange(H)
    Dc = np.exp(-2j * math.pi * np.outer(n, n) / H) / math.sqrt(H)
    return Dc.real.astype(np.float32), Dc.imag.astype(np.float32)


def _build_nc():
    nc = bacc.Bacc("TRN2", target_bir_lowering=False)

    x_ext = nc.declare_dram_parameter("x", [C, HW], F32, isOutput=False)
    win_ext = nc.declare_dram_parameter("win", [2, 128, M], BF16, isOutput=False)
    wout_ext = nc.declare_dram_parameter("wout", [M, C], BF16, isOutput=False)
    dmat_ext = nc.declare_dram_parameter("dmat", [128, 3 * 128], BF16, isOutput=False)
    ident_ext = nc.declare_dram_parameter("ident", [128, 128], BF16, isOutput=False)
    gain_ext = nc.declare_dram_parameter("gain", [128, M * H], BF16, isOutput=False)
    out_ext = nc.declare_dram_parameter("out", [C, HW], F32, isOutput=True)

    with tile.TileContext(nc) as tc:
        with (
            tc.tile_pool(name="const", bufs=1) as cpool,
            tc.tile_pool(name="big", bufs=1) as bigpool,
            tc.tile_pool(name="work", bufs=3) as wpool,
        ):
            # ---- constants -------------------------------------------------
            dmat = cpool.tile([128, 3 * 128], BF16)
            nc.sync.dma_start(out=dmat[:], in_=dmat_ext[:])
            Dr, Di, Din = dmat[:, 0:128], dmat[:, 128:256], dmat[:, 256:384]
            ident = cpool.tile([128, 128], BF16)
            nc.sync.dma_start(out=ident[:], in_=ident_ext[:])
            win = cpool.tile([128, 2 * M], BF16)
            for ch in range(2):
                nc.sync.dma_start(out=win[:, ch * M:(ch + 1) * M], in_=win_ext[ch])
            wout = cpool.tile([M, C], BF16)
            nc.sync.dma_start(out=wout[:], in_=wout_ext[:])
            gain = cpool.tile([128, M * H], BF16)
            nc.sync.dma_start(out=gain[:], in_=gain_ext[:])

            # ---- big resident buffers -------------------------------------
            xb = bigpool.tile([128, 2 * HW], BF16)      # x in bf16, [c%128, (c//128, hw)]
            xt = bigpool.tile([128, M * H], BF16)       # x_proj^T  [w, (m, h)]
            xe = bigpool.tile([M, HW], BF16)            # x_enh     [m, (h, w)]
            sb = {
                n: bigpool.tile([128, M * H], BF16, name=n, tag=n)
                for n in ("ar", "ai", "atr", "ati", "yr", "yi",
                          "cr", "ci", "ctr", "cti", "zt", "zb")
            }

            # ---- phase 1: x in (SWDGE cast) + proj-in ----------------------
            for ch in range(2):
                for q in range(8):
                    nc.gpsimd.dma_start(
                        out=xb[:, ch * HW + q * 2048: ch * HW + (q + 1) * 2048],
                        in_=x_ext[ch * 128:(ch + 1) * 128, q * 2048:(q + 1) * 2048],
                    )

            with tc.tile_pool(name="psproj", bufs=1, space="PSUM") as pp:
                ps_proj = pp.tile([128, M * H], F32)     # 4 banks, [w, (h, m)]
                for ch in range(2):
                    for h in range(H):
                        nc.tensor.matmul(
                            ps_proj[:, h * M:(h + 1) * M],
                            lhsT=xb[:, ch * HW + h * W: ch * HW + (h + 1) * W],
                            rhs=win[:, ch * M:(ch + 1) * M],
                            start=(ch == 0 and h % 32 == 0),
                            stop=(ch == 1 and h % 32 == 31),
                            skip_group_check=True,
                        )
                # strided drain: [w,(h,m)] fp32 PSUM -> [w,(m,h)] bf16 SBUF
                nc.vector.tensor_copy(
                    out=xt[:].rearrange("p (m h) -> p m h", m=M),
                    in_=ps_proj[:].rearrange("p (h m) -> p m h", m=M),
                )

            # ---- phase 2: FFT chain (per channel-group of 4) ---------------
            # xt = x_proj^T [w, (m, h)].  Per channel:
            #   A = D @ X^T          [kw, h]   (contract w)
            #   AT = A^T             [h, kw]
            #   B = D @ AT = F       [kh, kw]  (contract h)
            #   Y = gain .* B        [kh, kw]
            #   C = conj(D) @ Y      [h, kw]
            #   CT = C^T             [kw, h]
            #   Zt = Re(conj(D)@CT)  [w, h]    = x_enh^T
            #   Z = Zt^T             [h, w]    = x_enh
            # Software-pipelined emission: engine queues are in-order, so a
            # per-group serial chain head-of-line-blocks the PE. Emitting
            # groups offset by OFS stages keeps ready PE work queued while
            # drains for other groups complete.
            with tc.tile_pool(name="psfft", bufs=4, space="PSUM") as pf:
                gt = [dict() for _ in range(NG)]

                def emit_stage(g, st):
                    gs = slice(g * GW, (g + 1) * GW)
                    T = gt[g]
                    if st == 0:      # A = D @ X^T
                        T["ar"] = pf.tile([128, GW], F32, name="psar", tag="psa")
                        T["ai"] = pf.tile([128, GW], F32, name="psai", tag="psb")
                        nc.tensor.matmul(T["ar"][:], lhsT=Dr, rhs=xt[:, gs])
                        nc.tensor.matmul(T["ai"][:], lhsT=Di, rhs=xt[:, gs])
                    elif st == 1:    # drain A
                        nc.scalar.copy(out=sb["ar"][:, gs], in_=T.pop("ar")[:])
                        nc.scalar.copy(out=sb["ai"][:, gs], in_=T.pop("ai")[:])
                    elif st == 2:    # AT transposes
                        T["tr"] = pf.tile([128, GW], BF16, name="pstr", tag="psa")
                        T["ti"] = pf.tile([128, GW], BF16, name="psti", tag="psb")
                        for k in range(4):
                            m0 = g * 4 + k
                            ks = slice(k * 128, (k + 1) * 128)
                            nc.tensor.transpose(
                                T["tr"][:, ks],
                                sb["ar"][:, m0 * 128:(m0 + 1) * 128], ident)
                            nc.tensor.transpose(
                                T["ti"][:, ks],
                                sb["ai"][:, m0 * 128:(m0 + 1) * 128], ident)
                    elif st == 3:    # AT copies
                        nc.vector.tensor_copy(out=sb["atr"][:, gs], in_=T.pop("tr")[:])
                        nc.vector.tensor_copy(out=sb["ati"][:, gs], in_=T.pop("ti")[:])
                    elif st == 4:    # B = D @ AT = F
                        T["br"] = pf.tile([128, GW], F32, name="psbr", tag="psa")
                        T["bi"] = pf.tile([128, GW], F32, name="psbi", tag="psb")
                        nc.tensor.matmul(T["br"][:], lhsT=Dr, rhs=sb["atr"][:, gs],
                                         start=True, stop=False)
                        nc.tensor.matmul(T["bi"][:], lhsT=Dr, rhs=sb["ati"][:, gs],
                                         start=True, stop=False)
                        nc.tensor.matmul(T["bi"][:], lhsT=Di, rhs=sb["atr"][:, gs],
                                         start=False, stop=True)
                        nc.tensor.matmul(T["br"][:], lhsT=Din, rhs=sb["ati"][:, gs],
                                         start=False, stop=True)
                    elif st == 5:    # gain
                        nc.vector.tensor_mul(out=sb["yr"][:, gs], in0=T.pop("br")[:],
                                             in1=gain[:, gs])
                        nc.vector.tensor_mul(out=sb["yi"][:, gs], in0=T.pop("bi")[:],
                                             in1=gain[:, gs])
                    elif st == 6:    # C = conj(D) @ Y
                        T["cr"] = pf.tile([128, GW], F32, name="pscr", tag="psa")
                        T["ci"] = pf.tile([128, GW], F32, name="psci", tag="psb")
                        nc.tensor.matmul(T["cr"][:], lhsT=Dr, rhs=sb["yr"][:, gs],
                                         start=True, stop=False)
                        nc.tensor.matmul(T["ci"][:], lhsT=Dr, rhs=sb["yi"][:, gs],
                                         start=True, stop=False)
                        nc.tensor.matmul(T["cr"][:], lhsT=Di, rhs=sb["yi"][:, gs],
                                         start=False, stop=True)
                        nc.tensor.matmul(T["ci"][:], lhsT=Din, rhs=sb["yr"][:, gs],
                                         start=False, stop=True)
                    elif st == 7:    # drain C
                        nc.scalar.copy(out=sb["cr"][:, gs], in_=T.pop("cr")[:])
                        nc.scalar.copy(out=sb["ci"][:, gs], in_=T.pop("ci")[:])
                    elif st == 8:    # CT transposes
                        T["ur"] = pf.tile([128, GW], BF16, name="psur", tag="psa")
                        T["ui"] = pf.tile([128, GW], BF16, name="psui", tag="psb")
                        for k in range(4):
                            m0 = g * 4 + k
                            ks = slice(k * 128, (k + 1) * 128)
                            nc.tensor.transpose(
                                T["ur"][:, ks],
                                sb["cr"][:, m0 * 128:(m0 + 1) * 128], ident)
                            nc.tensor.transpose(
                                T["ui"][:, ks],
                                sb["ci"][:, m0 * 128:(m0 + 1) * 128], ident)
                    elif st == 9:    # CT copies
                        nc.vector.tensor_copy(out=sb["ctr"][:, gs], in_=T.pop("ur")[:])
                        nc.vector.tensor_copy(out=sb["cti"][:, gs], in_=T.pop("ui")[:])
                    elif st == 10:   # Zt = Re(conj(D) @ CT) = x_enh^T
                        T["zt"] = pf.tile([128, GW], F32, name="pszt", tag="psa")
                        nc.tensor.matmul(T["zt"][:], lhsT=Dr, rhs=sb["ctr"][:, gs],
                                         start=True, stop=False)
                        nc.tensor.matmul(T["zt"][:], lhsT=Di, rhs=sb["cti"][:, gs],
                                         start=False, stop=True)
                    elif st == 11:   # drain Zt
                        nc.scalar.copy(out=sb["zt"][:, gs], in_=T.pop("zt")[:])
                    elif st == 12:   # final transpose -> x_enh [h, (m, w)]
                        T["z2"] = pf.tile([128, GW], BF16, name="psz2", tag="psb")
                        for k in range(4):
                            m0 = g * 4 + k
                            ks = slice(k * 128, (k + 1) * 128)
                            nc.tensor.transpose(
                                T["z2"][:, ks],
                                sb["zt"][:, m0 * 128:(m0 + 1) * 128], ident)
                    elif st == 13:   # drain Z
                        nc.scalar.copy(out=sb["zb"][:, gs], in_=T.pop("z2")[:])
                    elif st == 14:   # redistribute to xe [m, (h, w)]
                        for k in range(4):
                            m0 = g * 4 + k
                            nc.gpsimd.dma_start(
                                out=xe[m0:m0 + 1, :],
                                in_=sb["zb"][:, m0 * W:(m0 + 1) * W],
                            )

                N_ST, OFS = 15, 3
                for t in range(N_ST + OFS * (NG - 1)):
                    for g in range(NG - 1, -1, -1):
                        st = t - OFS * g
                        if 0 <= st < N_ST:
                            emit_stage(g, st)

            # ---- phase 4: proj-out + clip + residual + out DMA -------------
            # ch0: DVE drains PSUM with fused clip (tensor_scalar dual-op).
            # ch1: ACT drains, GPSIMD clips — balances the three engines.
            with tc.tile_pool(name="psout", bufs=1, space="PSUM") as po:
                for pg in range(8):          # 8 groups x 2048 pixels
                    pgs = slice(pg * 2048, (pg + 1) * 2048)
                    psp = [po.tile([128, 2048], F32, name=f"po{ch}", tag=f"po{ch}")
                           for ch in range(2)]
                    for ch in range(2):
                        for cc in range(4):
                            pc = pg * 4 + cc
                            nc.tensor.matmul(
                                psp[ch][:, cc * 512:(cc + 1) * 512],
                                lhsT=wout[:, ch * 128:(ch + 1) * 128],
                                rhs=xe[:, pc * 512:(pc + 1) * 512],
                            )
                    pb0 = wpool.tile([128, 2048], BF16, tag="pb0")
                    nc.vector.tensor_scalar(
                        out=pb0[:], in0=psp[0][:],
                        scalar1=-OUT_CLIP, scalar2=OUT_CLIP,
                        op0=mybir.AluOpType.max, op1=mybir.AluOpType.min,
                    )
                    pb1 = wpool.tile([128, 2048], BF16, tag="pb1")
                    nc.scalar.copy(out=pb1[:], in_=psp[1][:])
                    nc.vector.tensor_scalar(
                        out=pb1[:], in0=pb1[:],
                        scalar1=-OUT_CLIP, scalar2=OUT_CLIP,
                        op0=mybir.AluOpType.max, op1=mybir.AluOpType.min,
                    )
                    for ch, pb in ((0, pb0), (1, pb1)):
                        xs = xb[:, ch * HW + pg * 2048: ch * HW + (pg + 1) * 2048]
                        nc.vector.tensor_add(out=xs, in0=xs, in1=pb[:])
                        nc.gpsimd.dma_start(
                            out=out_ext[ch * 128:(ch + 1) * 128, pgs],
                            in_=xs,
                        )
    nc.finalize()
    return nc


_NC = None


def _get_nc():
    global _NC
    if _NC is None:
        _NC = _build_nc()
    return _NC


def _consts(w_in, w_out, weights_raw):
    Drm, Dim = _dft_mats()
    dmat = np.concatenate([Drm, Dim, -Dim], axis=1).astype(BF)
    ident = np.eye(128, dtype=BF)
    win = np.ascontiguousarray(
        np.asarray(w_in, np.float32).T.reshape(2, 128, M)).astype(BF)
    wout = np.ascontiguousarray(np.asarray(w_out, np.float32).T).astype(BF)
    gain = _build_gain(np.asarray(weights_raw, np.float32)).astype(BF)
    return dmat, ident, win, wout, gain


def kernel(x, w_in, w_out, weights_raw):
    x = np.asarray(x, np.float32)
    dmat, ident, win, wout, gain = _consts(w_in, w_out, weights_raw)
    nc = _get_nc()
    in_maps = [
        {
            "x": np.ascontiguousarray(x[b].reshape(C, HW)),
            "win": win,
            "wout": wout,
            "dmat": dmat,
            "ident": ident,
            "gain": gain,
        }
        for b in range(B)
    ]
    res = run_bass_kernel_spmd(nc, in_maps, core_ids=list(range(N_CORES)))
    out = np.stack([np.asarray(res.results[b]["out"], np.float32) for b in range(B)])
    return out.reshape(B, C, H, W)


# revision 32
# speedup vs baseline: 1.3152x; 1.0377x over previous
"""Trainium2 Bass kernel for nn_AngleFreqEnhance (8-core data-parallel).

Math: out = x + clip(w_out @ Re(IFFT2(gain * FFT2(w_in @ x))), -10, 10)

Key facts exploited:
  * f_enh = (|f|+eps)*gain*exp(i*angle(f)) == gain*f up to O(eps)=1e-8 — the
    frequency step is a pointwise REAL gain multiply.
  * gain depends only on weights_raw (tiny, replicated) — computed on host,
    bit-exact vs the reference via jax-on-CPU (XLA lowers `(t+pi)%pi` to an
    IEEE remainder; numpy does not match it).
  * FFT2/IFFT2 on 128x128 become matmuls with the (symmetric, ortho) DFT
    matrix D: F = D X D. All heavy lifting is TensorEngine bf16 matmuls with
    fp32 PSUM accumulate; rel_l2 error of the whole pipeline ~6e-3.

Per-core dataflow (one sample, B=8 == 8 cores, no collectives):
  DMA-cast x fp32->bf16 (SWDGE) -> proj-in (data-as-weights matmuls,
  out = x_proj^T in [w,(m,h)]) -> per-channel transpose-chain FFT,
  software-pipelined across 4 channel-groups (engine queues are in-order,
  so stages of different groups are emitted interleaved):
      A=D@XT; AT=A^T; B=D@AT=F; Y=gain.*F; C=conj(D)@Y; CT=C^T;
      Zt=Re(conj(D)@CT)=x_enh^T; Z=Zt^T=x_enh
  -> per-channel SBUF->SBUF HWDGE DMA redistribution to xe [m=16, pixels]
  -> proj-out (K=16 matmuls, double-buffered PSUM) -> drains (DVE fused
  clip for ch0, ACT copy + DVE clip for ch1) -> DVE residual add into
  x_bf16 in place -> SWDGE DMA-cast bf16->fp32 out.

Measured on 8x TRN2 cores: ~146-170 us vs ~94 us DMA roofline
(in 45 + FFT ~30 + out 47 + fixed overheads); rel_l2 err 6.2e-3.
"""
import math

import numpy as np
import ml_dtypes

import concourse.bacc as bacc
import concourse.mybir as mybir
import concourse.tile as tile
from concourse.bass_utils import run_bass_kernel_spmd

B, C, H, W = 8, 256, 128, 128
M = 16                    # mid channels
HW = H * W                # 16384
N_CORES = 8
OUT_CLIP = 10.0
EPS = 1e-8
N_ANGLES, RADIUS_WIDTH, N_RADII = 8, 8, 9
OVERLAP, HF_RATIO = 1.5, 0.3

F32 = mybir.dt.float32
BF16 = mybir.dt.bfloat16
BF = ml_dtypes.bfloat16

NG = 4                    # channel groups for the FFT chain
GW = (M // NG) * H        # free width of one group = 512


def _build_gain(weights_raw: np.ndarray) -> np.ndarray:
    """Replicates reference gain computation bit-exactly (jax on CPU), returns
    [128, M*128] fp32 laid out [k_h, (m, k_w)] in UNSHIFTED freq coords."""
    import jax
    import jax.numpy as jnp

    cpu = jax.devices("cpu")[0]
    with jax.default_device(cpu):
        cy, cx = H // 2, W // 2
        yy = jnp.arange(H, dtype=jnp.float32)[:, None] - cy
        xx = jnp.arange(W, dtype=jnp.float32)[None, :] - cx
        r = jnp.sqrt(yy * yy + xx * xx)
        theta = (jnp.arctan2(yy, xx) + math.pi) % math.pi
        radius_idx = jnp.clip(
            jnp.floor(r / RADIUS_WIDTH).astype(jnp.int32), 0, N_RADII - 1
        )
        delta = math.pi / N_ANGLES
        half_width = OVERLAP * delta / 2.0
        centers = (jnp.arange(N_ANGLES, dtype=jnp.float32) + 0.5) * delta
        dist = jnp.abs(theta[None, :, :] - centers[:, None, None])
        aw = jnp.clip(1.0 - dist / half_width, 0.0) * (dist < half_width)
        aw = aw / (aw.sum(axis=0, keepdims=True) + EPS)
        max_r = float(max(cy, cx))
        high = (r > HF_RATIO * max_r) if HF_RATIO > 0 else jnp.ones_like(r, dtype=bool)
        valid = (r >= 0.5) & high
        wt = 1.0 + jnp.tanh(jnp.asarray(weights_raw, dtype=jnp.float32))
        w_pix = wt[:, :, radius_idx]
        gain = jnp.einsum("ahw,mahw->mhw", aw, w_pix)
        gain = jnp.where(valid[None], gain, 1.0)
        gain = np.asarray(gain, dtype=np.float32)
    gain = np.fft.ifftshift(gain, axes=(-2, -1))          # [m, kh, kw]
    # per-channel layout [kh, (m, kw)] to match F in the chain
    return np.ascontiguousarray(gain.transpose(1, 0, 2).reshape(H, M * W))


def _dft_mats():
    n = np.arange(H)
    Dc = np.exp(-2j * math.pi * np.outer(n, n) / H) / math.sqrt(H)
    return Dc.real.astype(np.float32), Dc.imag.astype(np.float32)


def _build_nc():
    nc = bacc.Bacc("TRN2", target_bir_lowering=False)

    x_ext = nc.declare_dram_parameter("x", [C, HW], F32, isOutput=False)
    win_ext = nc.declare_dram_parameter("win", [2, 128, M], BF16, isOutput=False)
    wout_ext = nc.declare_dram_parameter("wout", [M, C], BF16, isOutput=False)
    dmat_ext = nc.declare_dram_parameter("dmat", [128, 3 * 128], BF16, isOutput=False)
    ident_ext = nc.declare_dram_parameter("ident", [128, 128], BF16, isOutput=False)
    gain_ext = nc.declare_dram_parameter("gain", [128, M * H], BF16, isOutput=False)
    out_ext = nc.declare_dram_parameter("out", [C, HW], F32, isOutput=True)

    with tile.TileContext(nc) as tc:
        with (
            tc.tile_pool(name="const", bufs=1) as cpool,
            tc.tile_pool(name="big", bufs=1) as bigpool,
            tc.tile_pool(name="work", bufs=3) as wpool,
        ):
            # ---- constants -------------------------------------------------
            dmat = cpool.tile([128, 3 * 128], BF16)
            nc.sync.dma_start(out=dmat[:], in_=dmat_ext[:])
            Dr, Di, Din = dmat[:, 0:128], dmat[:, 128:256], dmat[:, 256:384]
            ident = cpool.tile([128, 128], BF16)
            nc.sync.dma_start(out=ident[:], in_=ident_ext[:])
            win = cpool.tile([128, 2 * M], BF16)
            for ch in range(2):
                nc.sync.dma_start(out=win[:, ch * M:(ch + 1) * M], in_=win_ext[ch])
            wout = cpool.tile([M, C], BF16)
            nc.sync.dma_start(out=wout[:], in_=wout_ext[:])
            gain = cpool.tile([128, M * H], BF16)
            nc.sync.dma_start(out=gain[:], in_=gain_ext[:])

            # ---- big resident buffers -------------------------------------
            xb = bigpool.tile([128, 2 * HW], BF16)      # x in bf16, [c%128, (c//128, hw)]
            xt = bigpool.tile([128, M * H], BF16)       # x_proj^T  [w, (m, h)]
            xe = bigpool.tile([M, HW], BF16)            # x_enh     [m, (h, w)]
            sb = {
                n: bigpool.tile([128, M * H], BF16, name=n, tag=n)
                for n in ("ar", "ai", "atr", "ati", "yr", "yi",
                          "cr", "ci", "ctr", "cti", "zt", "zb")
            }

            # ---- phase 1: x in (SWDGE cast) ------------------------------
            for ch in range(2):
                for q in range(8):
                    nc.gpsimd.dma_start(
                        out=xb[:, ch * HW + q * 2048: ch * HW + (q + 1) * 2048],
                        in_=x_ext[ch * 128:(ch + 1) * 128, q * 2048:(q + 1) * 2048],
                    )

            with tc.tile_pool(name="psproj", bufs=1, space="PSUM") as pp:
                ps_proj = pp.tile([128, M * H], F32)     # 4 banks, [w, (h, m)]
                for ch in range(2):
                    for h in range(H):
                        nc.tensor.matmul(
                            ps_proj[:, h * M:(h + 1) * M],
                            lhsT=xb[:, ch * HW + h * W: ch * HW + (h + 1) * W],
                            rhs=win[:, ch * M:(ch + 1) * M],
                            start=(ch == 0 and h % 32 == 0),
                            stop=(ch == 1 and h % 32 == 31),
                            skip_group_check=True,
                        )
                # strided drain: [w,(h,m)] fp32 PSUM -> [w,(m,h)] bf16 SBUF
                nc.vector.tensor_copy(
                    out=xt[:].rearrange("p (m h) -> p m h", m=M),
                    in_=ps_proj[:].rearrange("p (h m) -> p m h", m=M),
                )

            # ---- phase 2: FFT chain (per channel-group of 4) ---------------
            # xt = x_proj^T [w, (m, h)].  Per channel:
            #   A = D @ X^T          [kw, h]   (contract w)
            #   AT = A^T             [h, kw]
            #   B = D @ AT = F       [kh, kw]  (contract h)
            #   Y = gain .* B        [kh, kw]
            #   C = conj(D) @ Y      [h, kw]
            #   CT = C^T             [kw, h]
            #   Zt = Re(conj(D)@CT)  [w, h]    = x_enh^T
            #   Z = Zt^T             [h, w]    = x_enh
            # Software-pipelined emission: engine queues are in-order, so a
            # per-group serial chain head-of-line-blocks the PE. Emitting
            # groups offset by OFS stages keeps ready PE work queued while
            # drains for other groups complete.
            with tc.tile_pool(name="psfft", bufs=4, space="PSUM") as pf:
                gt = [dict() for _ in range(NG)]

                def emit_stage(g, st):
                    gs = slice(g * GW, (g + 1) * GW)
                    T = gt[g]
                    CPG = M // NG      # channels per group
                    if st == 0:      # A = D @ X^T
                        T["ar"] = pf.tile([128, GW], F32, name="psar", tag="psa")
                        T["ai"] = pf.tile([128, GW], F32, name="psai", tag="psb")
                        nc.tensor.matmul(T["ar"][:], lhsT=Dr, rhs=xt[:, gs])
                        nc.tensor.matmul(T["ai"][:], lhsT=Di, rhs=xt[:, gs])
                    elif st == 1:    # drain A
                        nc.scalar.copy(out=sb["ar"][:, gs], in_=T.pop("ar")[:])
                        nc.scalar.copy(out=sb["ai"][:, gs], in_=T.pop("ai")[:])
                    elif st == 2:    # AT transposes
                        T["tr"] = pf.tile([128, GW], BF16, name="pstr", tag="psa")
                        T["ti"] = pf.tile([128, GW], BF16, name="psti", tag="psb")
                        for k in range(CPG):
                            m0 = g * CPG + k
                            ks = slice(k * 128, (k + 1) * 128)
                            nc.tensor.transpose(
                                T["tr"][:, ks],
                                sb["ar"][:, m0 * 128:(m0 + 1) * 128], ident)
                            nc.tensor.transpose(
                                T["ti"][:, ks],
                                sb["ai"][:, m0 * 128:(m0 + 1) * 128], ident)
                    elif st == 3:    # AT copies
                        nc.vector.tensor_copy(out=sb["atr"][:, gs], in_=T.pop("tr")[:])
                        nc.vector.tensor_copy(out=sb["ati"][:, gs], in_=T.pop("ti")[:])
                    elif st == 4:    # B = D @ AT = F
                        T["br"] = pf.tile([128, GW], F32, name="psbr", tag="psa")
                        T["bi"] = pf.tile([128, GW], F32, name="psbi", tag="psb")
                        nc.tensor.matmul(T["br"][:], lhsT=Dr, rhs=sb["atr"][:, gs],
                                         start=True, stop=False)
                        nc.tensor.matmul(T["bi"][:], lhsT=Dr, rhs=sb["ati"][:, gs],
                                         start=True, stop=False)
                        nc.tensor.matmul(T["bi"][:], lhsT=Di, rhs=sb["atr"][:, gs],
                                         start=False, stop=True)
                        nc.tensor.matmul(T["br"][:], lhsT=Din, rhs=sb["ati"][:, gs],
                                         start=False, stop=True)
                    elif st == 5:    # gain
                        nc.vector.tensor_mul(out=sb["yr"][:, gs], in0=T.pop("br")[:],
                                             in1=gain[:, gs])
                        nc.vector.tensor_mul(out=sb["yi"][:, gs], in0=T.pop("bi")[:],
                                             in1=gain[:, gs])
                    elif st == 6:    # C = conj(D) @ Y
                        T["cr"] = pf.tile([128, GW], F32, name="pscr", tag="psa")
                        T["ci"] = pf.tile([128, GW], F32, name="psci", tag="psb")
                        nc.tensor.matmul(T["cr"][:], lhsT=Dr, rhs=sb["yr"][:, gs],
                                         start=True, stop=False)
                        nc.tensor.matmul(T["ci"][:], lhsT=Dr, rhs=sb["yi"][:, gs],
                                         start=True, stop=False)
                        nc.tensor.matmul(T["cr"][:], lhsT=Di, rhs=sb["yi"][:, gs],
                                         start=False, stop=True)
                        nc.tensor.matmul(T["ci"][:], lhsT=Din, rhs=sb["yr"][:, gs],
                                         start=False, stop=True)
                    elif st == 7:    # drain C
                        nc.scalar.copy(out=sb["cr"][:, gs], in_=T.pop("cr")[:])
                        nc.scalar.copy(out=sb["ci"][:, gs], in_=T.pop("ci")[:])
                    elif st == 8:    # CT transposes
                        T["ur"] = pf.tile([128, GW], BF16, name="psur", tag="psa")
                        T["ui"] = pf.tile([128, GW], BF16, name="psui", tag="psb")
                        for k in range(CPG):
                            m0 = g * CPG + k
                            ks = slice(k * 128, (k + 1) * 128)
                            nc.tensor.transpose(
                                T["ur"][:, ks],
                                sb["cr"][:, m0 * 128:(m0 + 1) * 128], ident)
                            nc.tensor.transpose(
                                T["ui"][:, ks],
                                sb["ci"][:, m0 * 128:(m0 + 1) * 128], ident)
                    elif st == 9:    # CT copies
                        nc.vector.tensor_copy(out=sb["ctr"][:, gs], in_=T.pop("ur")[:])
                        nc.vector.tensor_copy(out=sb["cti"][:, gs], in_=T.pop("ui")[:])
                    elif st == 10:   # Zt = Re(conj(D) @ CT) = x_enh^T
                        T["zt"] = pf.tile([128, GW], F32, name="pszt", tag="psa")
                        nc.tensor.matmul(T["zt"][:], lhsT=Dr, rhs=sb["ctr"][:, gs],
                                         start=True, stop=False)
                        nc.tensor.matmul(T["zt"][:], lhsT=Di, rhs=sb["cti"][:, gs],
                                         start=False, stop=True)
                    elif st == 11:   # drain Zt
                        nc.scalar.copy(out=sb["zt"][:, gs], in_=T.pop("zt")[:])
                    elif st == 12:   # final transpose -> x_enh [h, (m, w)]
                        T["z2"] = pf.tile([128, GW], BF16, name="psz2", tag="psb")
                        for k in range(CPG):
                            m0 = g * CPG + k
                            ks = slice(k * 128, (k + 1) * 128)
                            nc.tensor.transpose(
                                T["z2"][:, ks],
                                sb["zt"][:, m0 * 128:(m0 + 1) * 128], ident)
                    elif st == 13:   # drain Z per channel + immediate redist
                        z2 = T.pop("z2")
                        for k in range(CPG):
                            m0 = g * CPG + k
                            ks = slice(k * 128, (k + 1) * 128)
                            nc.scalar.copy(
                                out=sb["zb"][:, m0 * 128:(m0 + 1) * 128],
                                in_=z2[:, ks])
                            # alternate the two HWDGE rings (SP / ACT)
                            eng = nc.sync if k % 2 == 0 else nc.scalar
                            eng.dma_start(
                                out=xe[m0:m0 + 1, :],
                                in_=sb["zb"][:, m0 * W:(m0 + 1) * W],
                            )
                    elif st == 14:   # (folded into stage 13)
                        pass

                N_ST, OFS = 15, 3
                for t in range(N_ST + OFS * (NG - 1)):
                    for g in range(NG - 1, -1, -1):
                        st = t - OFS * g
                        if 0 <= st < N_ST:
                            emit_stage(g, st)

            # ---- phase 4: proj-out + clip + residual + out DMA -------------
            # ch0: DVE drains PSUM with fused clip (tensor_scalar dual-op).
            # ch1: ACT drains, GPSIMD clips — balances the three engines.
            with tc.tile_pool(name="psout", bufs=2, space="PSUM") as po:
                PGW = 1024
                for pg in range(HW // PGW):   # 16 groups x 1024 pixels
                    pgs = slice(pg * PGW, (pg + 1) * PGW)
                    psp = [po.tile([128, PGW], F32, name=f"po{ch}", tag=f"po{ch}")
                           for ch in range(2)]
                    for ch in range(2):
                        for cc in range(PGW // 512):
                            pc = pg * (PGW // 512) + cc
                            nc.tensor.matmul(
                                psp[ch][:, cc * 512:(cc + 1) * 512],
                                lhsT=wout[:, ch * 128:(ch + 1) * 128],
                                rhs=xe[:, pc * 512:(pc + 1) * 512],
                            )
                    pb0 = wpool.tile([128, PGW], BF16, tag="pb0")
                    nc.vector.tensor_scalar(
                        out=pb0[:], in0=psp[0][:],
                        scalar1=-OUT_CLIP, scalar2=OUT_CLIP,
                        op0=mybir.AluOpType.max, op1=mybir.AluOpType.min,
                    )
                    pb1 = wpool.tile([128, PGW], BF16, tag="pb1")
                    nc.scalar.copy(out=pb1[:], in_=psp[1][:])
                    nc.vector.tensor_scalar(
                        out=pb1[:], in0=pb1[:],
                        scalar1=-OUT_CLIP, scalar2=OUT_CLIP,
                        op0=mybir.AluOpType.max, op1=mybir.AluOpType.min,
                    )
                    for ch, pb in ((0, pb0), (1, pb1)):
                        xs = xb[:, ch * HW + pg * PGW: ch * HW + (pg + 1) * PGW]
                        nc.vector.tensor_add(out=xs, in0=xs, in1=pb[:])
                        nc.gpsimd.dma_start(
                            out=out_ext[ch * 128:(ch + 1) * 128, pgs],
                            in_=xs,
                        )
    nc.finalize()
    return nc


_NC = None


def _get_nc():
    global _NC
    if _NC is None:
        _NC = _build_nc()
    return _NC


def _consts(w_in, w_out, weights_raw):
    Drm, Dim = _dft_mats()
    dmat = np.concatenate([Drm, Dim, -Dim], axis=1).astype(BF)
    ident = np.eye(128, dtype=BF)
    win = np.ascontiguousarray(
        np.asarray(w_in, np.float32).T.reshape(2, 128, M)).astype(BF)
    wout = np.ascontiguousarray(np.asarray(w_out, np.float32).T).astype(BF)
    gain = _build_gain(np.asarray(weights_raw, np.float32)).astype(BF)
    return dmat, ident, win, wout, gain


def kernel(x, w_in, w_out, weights_raw):
    x = np.asarray(x, np.float32)
    dmat, ident, win, wout, gain = _consts(w_in, w_out, weights_raw)
    nc = _get_nc()
    in_maps = [
        {
            "x": np.ascontiguousarray(x[b].reshape(C, HW)),
            "win": win,
            "wout": wout,
            "dmat": dmat,
            "ident": ident,
            "gain": gain,
        }
        for b in range(B)
    ]
    res = run_bass_kernel_spmd(nc, in_maps, core_ids=list(range(N_CORES)))
    out = np.stack([np.asarray(res.results[b]["out"], np.float32) for b in range(B)])
    return out.reshape(B, C, H, W)
